# revision 1
# baseline (speedup 1.0000x reference)
"""Trainium2 Bass kernel for nn_NumDualDescriptorAB (sliding-window descriptor).

Reference computation:
    X = vec_seq @ M.T                       # [S, m]
    T[w] = mean_{r<rank} X[w+r]             # sliding window mean, W = S-rank+1
    j = w % L
    scalar[w] = Bbasis[j] . T[w]
    out[w]    = Acoeff.T[j] * scalar[w]

Algebraic rewrite used here (matmul is linear, dot distributes over the
window sum):
    C = Bbasis @ M                          # [L, m]  tiny - host precompute
    P[w] = sum_{r<rank} vec_seq[w+r]        # window *sum* of raw input rows
    scalar[w] = (1/rank) * (C[j] . P[w])
    out[w]    = Acoeff.T[j] * scalar[w]

This removes the big [S,m]x[m,m] matmul entirely. Device work per tile of
128 windows:
  - PE: banded 0/1-weight matmuls compute P (window sums across partitions)
  - DVE: fused tensor_tensor_reduce gives scalar[w] = (1/rank)*C[j].P[w]
  - ACT: activation(Copy, scale=scalar) broadcasts Acoeff.T rows * scalar
Sharded across 8 cores along the window axis; halo handled host-side by
overlapping shards (no collectives).
"""

import os

import numpy as np

import concourse.bacc as bacc
import concourse.bass as bass  # noqa: F401  (AP types etc.)
import concourse.mybir as mybir
import concourse.tile as tile
from concourse.bass_utils import run_bass_kernel_spmd

N_CORES = 8
M_DIM = 512  # vector dim m (= free dim of every tile)
L_DIM = 512  # number of basis rows; window w uses row w % L_DIM
SEQ = 131072
CHUNK = 128  # rows per loaded chunk == windows per tile

# "fp32": exact fp32 band matmuls (PE 4 cyc/row, no casts)
# "trunc": single bf16 band matmuls reading a zero-cost truncated-bf16
#          strided view of the fp32 chunk (~3e-3 rel err, no cast ops)
# "hilo": trunc-hi matmuls + GPSIMD residual lo matmuls (~1e-5 rel err)
MM_MODE = os.environ.get("KERNEL_MM_MODE", "hilo")

_NC_CACHE = {}
_LAST_RESULTS = None  # BassKernelResults of the most recent run (for test.py)


def build_nc(ntiles: int, rank: int, mode: str) -> bass.Bass:
    f32 = mybir.dt.float32
    bf16 = mybir.dt.bfloat16
    mm_dt = f32 if mode == "fp32" else bf16
    halo = rank - 1

    # Bacc (not raw Bass): its compile() pipeline splits multi-wait
    # instructions (TRN2 allows 1 sync wait per instruction) via
    # generate_event_semaphores; raw Bass programs fail walrus codegen.
    nc = bacc.Bacc()
    nrows = (ntiles + 1) * CHUNK
    v_d = nc.dram_tensor("v", [nrows, M_DIM], f32, kind="ExternalInput")
    c_d = nc.dram_tensor("cmat", [4, CHUNK, M_DIM], f32, kind="ExternalInput")
    a_d = nc.dram_tensor("amat", [4, CHUNK, M_DIM], f32, kind="ExternalInput")
    w1_d = nc.dram_tensor("w1", [CHUNK, CHUNK], mm_dt, kind="ExternalInput")
    if halo > 0:
        w2_d = nc.dram_tensor("w2", [halo, CHUNK], mm_dt, kind="ExternalInput")
    o_d = nc.dram_tensor("o", [ntiles * CHUNK, M_DIM], f32, kind="ExternalOutput")

    mult = mybir.AluOpType.mult
    add = mybir.AluOpType.add
    copy_f = mybir.ActivationFunctionType.Copy

    with tile.TileContext(nc) as tc:
        with (
            tc.tile_pool(name="consts", bufs=1) as consts,
            tc.tile_pool(name="chunks", bufs=8) as chunks,
            tc.tile_pool(name="casts", bufs=8) as casts,
            tc.tile_pool(name="psump", bufs=8, space="PSUM") as psump,
            tc.tile_pool(name="work", bufs=4) as work,
        ):
            c4 = consts.tile([CHUNK, 4, M_DIM], f32, tag="c4")
            a4 = consts.tile([CHUNK, 4, M_DIM], f32, tag="a4")
            w1t = consts.tile([CHUNK, CHUNK], mm_dt, tag="w1")
            for h in range(4):
                nc.sync.dma_start(out=c4[:, h, :], in_=c_d[h])
                nc.sync.dma_start(out=a4[:, h, :], in_=a_d[h])
            nc.sync.dma_start(out=w1t, in_=w1_d[:])
            if halo > 0:
                w2t = consts.tile([halo, CHUNK], mm_dt, tag="w2")
                nc.sync.dma_start(out=w2t, in_=w2_d[:])

            def load_chunk(t):
                """DMA chunk t and produce the matmul-ready views of it."""
                ch = chunks.tile([CHUNK, M_DIM], f32, tag="chunk")
                nc.sync.dma_start(out=ch, in_=v_d[t * CHUNK : (t + 1) * CHUNK])
                if mode == "fp32":
                    return (ch[:],)
                # The high 16 bits of an fp32 ARE its truncated bf16: a
                # stride-2 view gives the hi operand with zero compute.
                hi = ch[:].bitcast(bf16)[:, 1::2]
                if mode == "trunc":
                    return (hi,)
                lo = casts.tile([CHUNK, M_DIM], bf16, tag="lo")
                nc.gpsimd.tensor_sub(lo, ch, hi)  # GPSIMD: keep DVE for mul+reduce
                return (hi, lo)

            # Software-pipelined prefetch: issue chunk DMAs (and their lo
            # residuals) PF tiles ahead in program order, so the PE's
            # matmul waits are pre-satisfied (late waits serialize
            # LDWEIGHTS+MATMUL and keep the PE clock-throttled).
            PF = 6
            parts = {}
            for t in range(min(PF, ntiles + 1)):
                parts[t] = load_chunk(t)
            for t in range(ntiles):
                pf = t + PF
                if pf <= ntiles and pf not in parts:
                    parts[pf] = load_chunk(pf)
                prev = parts[t]
                nxt = parts[t + 1]
                ps = psump.tile([CHUNK, M_DIM], f32, tag="ps")
                n_mm = len(prev) * (2 if halo > 0 else 1)
                i_mm = 0
                for part in prev:
                    nc.tensor.matmul(
                        ps, w1t, part, start=(i_mm == 0), stop=(i_mm == n_mm - 1)
                    )
                    i_mm += 1
                if halo > 0:
                    for part in nxt:
                        nc.tensor.matmul(
                            ps,
                            w2t,
                            part[0:halo, :],
                            start=(i_mm == 0),
                            stop=(i_mm == n_mm - 1),
                        )
                        i_mm += 1

                # (tensor_tensor_reduce would fuse these, but that opcode
                # crashes the exec unit on this runtime - verified by probe.)
                # The 1/rank scale is folded into C host-side.
                sc = work.tile([CHUNK, M_DIM], f32, tag="sc")
                nc.vector.tensor_tensor(sc, ps, c4[:, t % 4, :], mult)
                s = work.tile([CHUNK, 1], f32, tag="s")
                nc.vector.tensor_reduce(s, sc, mybir.AxisListType.X, add)
                ot = work.tile([CHUNK, M_DIM], f32, tag="ot")
                nc.scalar.activation(out=ot, in_=a4[:, t % 4, :], func=copy_f, scale=s)
                nc.sync.dma_start(out=o_d[t * CHUNK : (t + 1) * CHUNK], in_=ot)
                del parts[t]

    nc.finalize()
    return nc


def _get_nc(ntiles: int, rank: int, mode: str) -> bass.Bass:
    key = (ntiles, rank, mode)
    if key not in _NC_CACHE:
        _NC_CACHE[key] = build_nc(ntiles, rank, mode)
    return _NC_CACHE[key]


def make_band_weights(rank: int, dtype):
    """W1[k,w]=1 iff row k of the chunk is inside window w (w<=k<=w+rank-1);
    W2[k,w]=1 iff row k of the *next* chunk is inside window w."""
    w1 = np.zeros((CHUNK, CHUNK), dtype=dtype)
    for k in range(CHUNK):
        w1[k, max(0, k - (rank - 1)) : k + 1] = 1
    halo = rank - 1
    w2 = np.zeros((max(halo, 1), CHUNK), dtype=dtype)
    for k in range(halo):
        w2[k, CHUNK - halo + k :] = 1
    return w1, w2


def kernel(vec_seq, M, Acoeff, Bbasis, rank):
    global _LAST_RESULTS
    vec_seq = np.ascontiguousarray(np.asarray(vec_seq, dtype=np.float32))
    M = np.asarray(M, dtype=np.float32)
    Acoeff = np.asarray(Acoeff, dtype=np.float32)
    Bbasis = np.asarray(Bbasis, dtype=np.float32)
    r = int(rank)
    S, m = vec_seq.shape
    assert m == M_DIM and Bbasis.shape[0] == L_DIM
    assert 1 <= r <= CHUNK

    W = S - r + 1  # number of windows
    # Per-core window count, padded to a multiple of CHUNK.
    ntiles = -(-W // (N_CORES * CHUNK))
    nw = ntiles * CHUNK
    nrows = (ntiles + 1) * CHUNK

    # Host-side parameter precompute (tiny: 512^3 matmul). The 1/rank
    # window-mean scale is folded into C.
    C = ((Bbasis.astype(np.float64) @ M.astype(np.float64)) / r).astype(np.float32)
    AT = np.ascontiguousarray(Acoeff.T).astype(np.float32)
    # Tile t uses basis rows j = (128*t .. 128*t+127) % 512 -> phase t%4.
    c4 = np.ascontiguousarray(C.reshape(4, CHUNK, M_DIM))
    a4 = np.ascontiguousarray(AT.reshape(4, CHUNK, M_DIM))

    mode = MM_MODE
    mm_np = np.float32 if mode == "fp32" else None
    w1, w2 = make_band_weights(r, np.float32)
    if mode != "fp32":
        import ml_dtypes

        w1 = w1.astype(ml_dtypes.bfloat16)
        w2 = w2.astype(ml_dtypes.bfloat16)

    nc = _get_nc(ntiles, r, mode)

    in_maps = []
    for k in range(N_CORES):
        lo = k * nw
        hi = min(S, lo + nrows)
        sh = np.zeros((nrows, M_DIM), dtype=np.float32)
        if hi > lo:
            sh[: hi - lo] = vec_seq[lo:hi]
        im = {"v": sh, "cmat": c4, "amat": a4, "w1": w1}
        if r > 1:
            im["w2"] = w2
        in_maps.append(im)

    res = run_bass_kernel_spmd(nc, in_maps, core_ids=list(range(N_CORES)))
    _LAST_RESULTS = res
    out = np.concatenate([res.results[k]["o"] for k in range(N_CORES)], axis=0)
    return np.ascontiguousarray(out[:W])

